# revision 39
# baseline (speedup 1.0000x reference)
"""Banded causal attention (local window 256) for trn2, 8-core SPMD.

Problem: B=2, H=16, S=2048, D=128, layer_idx=1 (odd) -> mask = causal AND
(j > i - 256). Each query attends to at most 256 keys, so scores are only
computed on the key-blocks (of 128) that intersect each query tile's
window.

Sharding: B*H = 32 head-slices, 4 per core.  Each core computes its heads'
full banded attention independently; the host merges heads afterwards.

Per-core kernel:
  - operands are fp16 (11-bit mantissa = same as the TF32 rounding the PE
    applies to fp32r matmuls, so accuracy is unchanged while DMA bytes and
    SBUF halve; fp16 matmuls run 1 cycle/row at any free dim and get fast
    weight loads)
  - host pre-transposes Q,K to [D, S] per head so no on-device transposes
  - per query-tile-pair (256 queries), scores S_T[kk, q] for the 4 key
    blocks that intersect; exp on ACT (scale=1/sqrt(D) folded in);
    triangular masks as 0/1 fp16 multiplies on DVE
  - ctx^T[d, q] and the softmax denominator accumulate in PSUM (fp32) via
    matmuls (lhsT = V tile / ones column); boundary blocks only compute
    their valid 128-column half
  - unnormalized fp16 ctx^T and fp32 denom DMA'd out; host divides and
    merges heads
"""

import math
import os
import sys

import numpy as np

for _p in ("/root/.axon_site/_ro/trn_rl_repo", "/opt/trn_rl_repo"):
    if os.path.isdir(_p) and _p not in sys.path:
        sys.path.append(_p)

import concourse.bacc as bacc
import concourse.mybir as mybir
import concourse.tile as tile
from concourse.bass_utils import run_bass_kernel_spmd

F32 = mybir.dt.float32
F16 = mybir.dt.float16

B, H, S, D = 2, 16, 2048, 128
P = 128
NT = S // P           # 16 query/key tiles per head-slice
NCORES = 8
G = (B * H) // NCORES  # 4 head-slices per core
WINDOW = 256
SCALE = 1.0 / math.sqrt(D)

_RUNNER_CACHE = {}


def build_nc():
    nc = bacc.Bacc("TRN2", target_bir_lowering=False, debug=False)
    qT = nc.declare_dram_parameter("qT", [G, P, S], F16, isOutput=False)
    kT = nc.declare_dram_parameter("kT", [G, P, S], F16, isOutput=False)
    # v is host-pre-tiled to [G, P, NT, D] (partition dim first) so the whole
    # head-slice loads as one fully-contiguous DMA
    v = nc.declare_dram_parameter("v", [G, P, NT, D], F16, isOutput=False)
    masks = nc.declare_dram_parameter("masks", [2, P, P], F16, isOutput=False)
    out_t = nc.declare_dram_parameter("out_t", [G, P, S], F16, isOutput=True)
    den = nc.declare_dram_parameter("den", [G, 1, S], F32, isOutput=True)

    EXP = mybir.ActivationFunctionType.Exp
    MUL = mybir.AluOpType.mult

    with tile.TileContext(nc) as tc:
        with (
            tc.tile_pool(name="const", bufs=1) as constp,
            tc.tile_pool(name="kv", bufs=3) as kvp,
            tc.tile_pool(name="pt", bufs=4) as ptp,
            tc.tile_pool(name="ps", bufs=2, space="PSUM") as psp,
        ):
            mhi = constp.tile([P, P], F16, tag="mhi")   # valid kk <= q
            nc.sync.dma_start(mhi, masks[0])
            mlo = constp.tile([P, P], F16, tag="mlo")   # valid kk > q
            nc.sync.dma_start(mlo, masks[1])
            ones = constp.tile([P, 1], F16, tag="ones")
            nc.vector.memset(ones, 1.0)

            for g in range(G):
                kt_sb = kvp.tile([P, NT, P], F16, tag="kt")
                qt_sb = kvp.tile([P, NT, P], F16, tag="qt")
                v_sb = kvp.tile([P, NT, D], F16, tag="v")
                # split across the two HWDGE rings (SP + ACT) plus SWDGE for
                # V so input loads run in parallel; halves let pair-0 compute
                # start before the whole head-slice has landed
                kt_d = kT[g].rearrange("d (n p) -> d n p", p=P)
                qt_d = qT[g].rearrange("d (n p) -> d n p", p=P)
                hn = NT // 2
                nc.sync.dma_start(kt_sb[:, 0:hn, :], kt_d[:, 0:hn, :])
                nc.scalar.dma_start(qt_sb[:, 0:hn, :], qt_d[:, 0:hn, :])
                nc.gpsimd.dma_start(v_sb[:, 0:hn, :], v[g][:, 0:hn, :])
                nc.sync.dma_start(kt_sb[:, hn:NT, :], kt_d[:, hn:NT, :])
                nc.scalar.dma_start(qt_sb[:, hn:NT, :], qt_d[:, hn:NT, :])
                nc.gpsimd.dma_start(v_sb[:, hn:NT, :], v[g][:, hn:NT, :])
                den_sb = kvp.tile([1, S], F32, tag="den")
                o_hs = kvp.tile([P, S], F16, tag="ohs")

                for pi in range(NT // 2):
                    t = 2 * pi            # first q-tile of the pair
                    q0 = t * P            # absolute first query column
                    # roles r=0..3 <-> key blocks t-2+r
                    # ([Mlo|--],[O|Mlo],[Mhi|O],[--|Mhi])
                    roles = [r for r in range(4) if t - 2 + r >= 0]
                    qs = qt_sb[:, t:t + 2, :].rearrange("d a b -> d (a b)")

                    ps03 = psp.tile([P, 4 * P], F32, tag="ps03")
                    ps12 = psp.tile([P, 4 * P], F32, tag="ps12")
                    psc = psp.tile([P, 2 * P], F32, tag="psc")
                    if pi % 2 == 0:
                        psd2 = psp.tile([1, 4 * P], F32, tag="psd", name="psd2")
                    psd = psd2[:, (pi % 2) * 2 * P:(pi % 2 + 1) * 2 * P]
                    ps0 = ps03[:, 0:2 * P]
                    ps3 = ps03[:, 2 * P:4 * P]

                    for r in roles:
                        kb = t - 2 + r
                        lhs = kt_sb[:, kb, :]
                        if r == 0:
                            tgt = ps0
                        elif r == 1:
                            tgt = ps12[:, 0:2 * P]
                        elif r == 2:
                            tgt = ps12[:, 2 * P:4 * P]
                        else:
                            tgt = ps3
                        nc.tensor.matmul(tgt, lhs, qs, start=True, stop=True)

                    # exp only on the valid halves; boundary roles 0/3 keep
                    # just their 128 valid columns (e0/e3 are [P, P])
                    e0 = (ptp.tile([P, P], F16, tag="e0", name="e0")
                          if 0 in roles else None)
                    e12 = ptp.tile([P, 4 * P], F16, tag="e12")
                    e3 = ptp.tile([P, P], F16, tag="e3")

                    if 0 in roles:
                        nc.scalar.activation(e0, ps0[:, 0:P], EXP, scale=SCALE)
                        nc.vector.tensor_tensor(e0, e0, mlo, MUL)
                    if 1 in roles:
                        nc.scalar.activation(e12, ps12, EXP, scale=SCALE)
                        nc.vector.tensor_tensor(
                            e12[:, P:2 * P], e12[:, P:2 * P], mlo, MUL)
                    else:
                        nc.scalar.activation(
                            e12[:, 2 * P:4 * P], ps12[:, 2 * P:4 * P], EXP,
                            scale=SCALE)
                    # role 2 (diagonal) and role 3 always present
                    nc.vector.tensor_tensor(
                        e12[:, 2 * P:3 * P], e12[:, 2 * P:3 * P], mhi, MUL)
                    nc.scalar.activation(e3, ps3[:, P:2 * P], EXP, scale=SCALE)
                    nc.vector.tensor_tensor(e3, e3, mhi, MUL)

                    # ctx^T and denominator accumulation; boundary roles only
                    # touch their valid half (N=128 runs full rate in fp16)
                    plan = []
                    for i, r in enumerate(roles):
                        kb = t - 2 + r
                        if r == 0:
                            rhs, csl, dsl = e0, slice(0, P), slice(0, P)
                        elif r == 1:
                            rhs, csl, dsl = (e12[:, 0:2 * P], slice(0, 2 * P),
                                             slice(0, 2 * P))
                        elif r == 2:
                            rhs, csl, dsl = (e12[:, 2 * P:4 * P],
                                             slice(0, 2 * P), slice(0, 2 * P))
                        else:
                            rhs, csl, dsl = e3, slice(P, 2 * P), slice(P, 2 * P)
                        plan.append((kb, rhs, csl, dsl))
                    # full-width roles first: the start=True matmul must cover
                    # the whole written range so PSUM pending-zero state stays
                    # uniform for the partial boundary matmuls that follow
                    plan.sort(key=lambda e: 0 if e[2] == slice(0, 2 * P) else 1)
                    for i, (kb, rhs, csl, dsl) in enumerate(plan):
                        first, last = i == 0, i == len(plan) - 1
                        nc.tensor.matmul(
                            psc[:, csl], v_sb[:, kb, :], rhs,
                            start=first, stop=last)
                        nc.tensor.matmul(
                            psd[:, dsl], ones, rhs, start=first, stop=last)

                    if pi % 2 == 0:
                        nc.scalar.copy(o_hs[:, q0:q0 + 2 * P], psc)
                    else:
                        nc.vector.tensor_copy(o_hs[:, q0:q0 + 2 * P], psc)
                    if pi % 2 == 1:
                        c0 = (pi - 1) * 2 * P
                        nc.vector.tensor_copy(den_sb[:, c0:c0 + 4 * P], psd2)
                        nc.scalar.dma_start(
                            out_t[g][:, c0:c0 + 4 * P], o_hs[:, c0:c0 + 4 * P])

                nc.sync.dma_start(den[g], den_sb)
    nc.compile()
    return nc


def _np_reference(q, k, v, layer_idx):
    """Slow fallback for an even layer_idx (pure causal) - not the graded
    configuration, kept for functional completeness."""
    scale = 1.0 / math.sqrt(q.shape[-1])
    s = np.einsum("bhqd,bhkd->bhqk", q, k) * scale
    i = np.arange(s.shape[-2])[:, None]
    j = np.arange(s.shape[-1])[None, :]
    mask = j <= i
    if layer_idx % 2 != 0:
        mask &= j > i - WINDOW
    s = np.where(mask[None, None], s, np.float32(-1e9))
    s -= s.max(-1, keepdims=True)
    w = np.exp(s)
    w /= w.sum(-1, keepdims=True)
    ctx = np.einsum("bhqk,bhkd->bhqd", w, v)
    b, h, sq, d = q.shape
    return ctx.transpose(0, 2, 1, 3).reshape(b, sq, h * d).astype(np.float32)


def make_in_maps(q, k, v):
    qf = q.reshape(B * H, S, D)
    kf = k.reshape(B * H, S, D)
    vf = v.reshape(B * H, S, D)
    qT = np.ascontiguousarray(qf.transpose(0, 2, 1)).astype(np.float16)
    kT = np.ascontiguousarray(kf.transpose(0, 2, 1)).astype(np.float16)
    # [BH, S, D] -> [BH, P, NT, D]: tile index inner so each head-slice's
    # V loads as one contiguous DMA into a [P, NT, D] SBUF tile
    vt = np.ascontiguousarray(
        vf.reshape(B * H, NT, P, D).transpose(0, 2, 1, 3)).astype(np.float16)

    m = np.zeros((2, P, P), dtype=np.float16)
    m[0] = np.triu(np.ones((P, P), np.float16))      # M_hi: kk <= q
    m[1] = np.tril(np.ones((P, P), np.float16), -1)  # M_lo: kk > q

    in_maps = []
    for c in range(NCORES):
        sl = slice(c * G, (c + 1) * G)
        in_maps.append({
            "qT": np.ascontiguousarray(qT[sl]),
            "kT": np.ascontiguousarray(kT[sl]),
            "v": np.ascontiguousarray(vt[sl]),
            "masks": m,
        })
    return in_maps


def kernel(q, k, v, layer_idx, training):
    q = np.asarray(q, dtype=np.float32)
    k = np.asarray(k, dtype=np.float32)
    v = np.asarray(v, dtype=np.float32)
    li = int(layer_idx)
    if li % 2 == 0:
        return _np_reference(q, k, v, li)

    in_maps = make_in_maps(q, k, v)

    if "nc" not in _RUNNER_CACHE:
        _RUNNER_CACHE["nc"] = build_nc()
    nc = _RUNNER_CACHE["nc"]
    res = run_bass_kernel_spmd(nc, in_maps, core_ids=list(range(NCORES)))

    ctx_t = np.concatenate(
        [r["out_t"] for r in res.results], axis=0).astype(np.float32)
    den = np.concatenate([r["den"] for r in res.results], axis=0)
    ctx_t = ctx_t / den                       # [32, D, S] / [32, 1, S]
    out = ctx_t.reshape(B, H, D, S).transpose(0, 3, 1, 2).reshape(B, S, H * D)
    return np.ascontiguousarray(out.astype(np.float32))


# revision 41
# speedup vs baseline: 1.1177x; 1.1177x over previous
"""Banded causal attention (local window 256) for trn2, 8-core SPMD.

Problem: B=2, H=16, S=2048, D=128, layer_idx=1 (odd) -> mask = causal AND
(j > i - 256). Each query attends to at most 256 keys, so scores are only
computed on the key-blocks (of 128) that intersect each query tile's
window.

Sharding: B*H = 32 head-slices, 4 per core.  Each core computes its heads'
full banded attention independently; the host merges heads afterwards.

Per-core kernel, per query-tile pair (256 queries, key blocks r0..r3):
  - fp16 operands everywhere (11-bit mantissa = the TF32 rounding the PE
    applies to fp32 anyway; halves DMA and SBUF; full matmul rate at any
    free dim)
  - scores S_T[kk, q]: r1/r2 at N=256, boundary r0/r3 only their valid
    128-query half; exp on ACT with scale=1/sqrt(D) folded in, written
    into one [128, 768] P tile; one 768-wide 0/1 mask multiply split
    between DVE and GpSimd
  - ctx^T[d, q] and softmax denominator accumulate in PSUM via matmuls
    (lhsT = V tile / ones column)
  - unnormalized fp16 ctx^T and fp32 denom DMA'd out; host divides and
    merges heads
"""

import math
import os
import sys

import numpy as np

for _p in ("/root/.axon_site/_ro/trn_rl_repo", "/opt/trn_rl_repo"):
    if os.path.isdir(_p) and _p not in sys.path:
        sys.path.append(_p)

import concourse.bacc as bacc
import concourse.mybir as mybir
import concourse.tile as tile
from concourse.bass_utils import run_bass_kernel_spmd

F32 = mybir.dt.float32
F16 = mybir.dt.float16

B, H, S, D = 2, 16, 2048, 128
P = 128
NT = S // P           # 16 query/key tiles per head-slice
NCORES = 8
G = (B * H) // NCORES  # 4 head-slices per core
WINDOW = 256
SCALE = 1.0 / math.sqrt(D)

_RUNNER_CACHE = {}


def build_nc():
    nc = bacc.Bacc("TRN2", target_bir_lowering=False, debug=False)
    qT = nc.declare_dram_parameter("qT", [G, P, S], F16, isOutput=False)
    kT = nc.declare_dram_parameter("kT", [G, P, S], F16, isOutput=False)
    # v is host-pre-tiled to [G, P, NT, D] (partition dim first) so the whole
    # head-slice loads as one fully-contiguous DMA
    v = nc.declare_dram_parameter("v", [G, P, NT, D], F16, isOutput=False)
    masks = nc.declare_dram_parameter("masks", [P, 6 * P], F16, isOutput=False)
    out_t = nc.declare_dram_parameter("out_t", [G, P, S], F16, isOutput=True)
    den = nc.declare_dram_parameter("den", [G, 1, S], F32, isOutput=True)

    EXP = mybir.ActivationFunctionType.Exp
    MUL = mybir.AluOpType.mult

    with tile.TileContext(nc) as tc:
        with (
            tc.tile_pool(name="const", bufs=1) as constp,
            tc.tile_pool(name="kv", bufs=3) as kvp,
            tc.tile_pool(name="pt", bufs=4) as ptp,
            tc.tile_pool(name="ps", bufs=2, space="PSUM") as psp,
            tc.tile_pool(name="ps1", bufs=1, space="PSUM") as psp1,
        ):
            # mask strip, columns [Mlo | 1 | Mlo | Mhi | 1 | Mhi] matching the
            # P-tile layout [r0(128) | r1(256) | r2(256) | r3(128)]
            strip = constp.tile([P, 6 * P], F16, tag="strip")
            nc.sync.dma_start(strip, masks.ap())
            ones = constp.tile([P, 1], F16, tag="ones")
            nc.vector.memset(ones, 1.0)

            for g in range(G):
                kt_sb = kvp.tile([P, NT, P], F16, tag="kt")
                qt_sb = kvp.tile([P, NT, P], F16, tag="qt")
                v_sb = kvp.tile([P, NT, D], F16, tag="v")
                # input loads: HWDGE(SP) for K/Q, SWDGE for V; halves so
                # pair-0 compute starts before the whole head-slice lands
                kt_d = kT[g].rearrange("d (n p) -> d n p", p=P)
                qt_d = qT[g].rearrange("d (n p) -> d n p", p=P)
                hn = NT // 2
                nc.sync.dma_start(kt_sb[:, 0:hn, :], kt_d[:, 0:hn, :])
                nc.sync.dma_start(qt_sb[:, 0:hn, :], qt_d[:, 0:hn, :])
                nc.gpsimd.dma_start(v_sb[:, 0:hn, :], v[g][:, 0:hn, :])
                nc.sync.dma_start(kt_sb[:, hn:NT, :], kt_d[:, hn:NT, :])
                nc.sync.dma_start(qt_sb[:, hn:NT, :], qt_d[:, hn:NT, :])
                nc.gpsimd.dma_start(v_sb[:, hn:NT, :], v[g][:, hn:NT, :])
                den_sb = kvp.tile([1, S], F32, tag="den")
                o_hs = kvp.tile([P, S], F16, tag="ohs")

                for pi in range(NT // 2):
                    t = 2 * pi            # first q-tile of the pair
                    q0 = t * P            # absolute first query column
                    roles = [r for r in range(4) if t - 2 + r >= 0]
                    qs = qt_sb[:, t:t + 2, :].rearrange("d a b -> d (a b)")

                    ps12 = psp.tile([P, 4 * P], F32, tag="ps12")
                    psc = psp.tile([P, 2 * P], F32, tag="psc")
                    if pi % 2 == 0:
                        psd2 = psp.tile([1, 4 * P], F32, tag="psd", name="psd2")
                    psd = psd2[:, (pi % 2) * 2 * P:(pi % 2 + 1) * 2 * P]
                    ps0 = (psp1.tile([P, P], F32, tag="ps0", name="ps0")
                           if 0 in roles else None)
                    ps3 = psp1.tile([P, P], F32, tag="ps3", name="ps3")

                    # score matmuls; boundary blocks only their valid q-half
                    if 0 in roles:
                        nc.tensor.matmul(ps0, kt_sb[:, t - 2, :], qs[:, 0:P],
                                         start=True, stop=True)
                    if 1 in roles:
                        nc.tensor.matmul(ps12[:, 0:2 * P], kt_sb[:, t - 1, :],
                                         qs, start=True, stop=True)
                    nc.tensor.matmul(ps12[:, 2 * P:4 * P], kt_sb[:, t, :],
                                     qs, start=True, stop=True)
                    nc.tensor.matmul(ps3, kt_sb[:, t + 1, :], qs[:, P:2 * P],
                                     start=True, stop=True)

                    # P tile [r0 | r1 | r2 | r3] = [128 | 256 | 256 | 128]
                    e = ptp.tile([P, 6 * P], F16, tag="e")
                    if 0 in roles:
                        nc.scalar.activation(e[:, 0:P], ps0, EXP, scale=SCALE)
                    if 1 in roles:
                        nc.scalar.activation(e[:, P:5 * P], ps12, EXP,
                                             scale=SCALE)
                    else:
                        nc.scalar.activation(e[:, 3 * P:5 * P],
                                             ps12[:, 2 * P:4 * P], EXP,
                                             scale=SCALE)
                    nc.scalar.activation(e[:, 5 * P:6 * P], ps3, EXP,
                                         scale=SCALE)

                    # one 0/1 mask multiply over the whole strip, split
                    # between DVE (first half) and GpSimd (second half)
                    if 0 in roles:
                        nc.vector.tensor_tensor(
                            e[:, 0:3 * P], e[:, 0:3 * P], strip[:, 0:3 * P],
                            MUL)
                    nc.gpsimd.tensor_tensor(
                        e[:, 3 * P:6 * P], e[:, 3 * P:6 * P],
                        strip[:, 3 * P:6 * P], MUL)

                    # ctx^T + denominator accumulation (full-width roles
                    # first so PSUM pending-zero state stays uniform)
                    plan = []
                    if 1 in roles:
                        plan.append((t - 1, e[:, P:3 * P], slice(0, 2 * P)))
                    plan.append((t, e[:, 3 * P:5 * P], slice(0, 2 * P)))
                    if 0 in roles:
                        plan.append((t - 2, e[:, 0:P], slice(0, P)))
                    plan.append((t + 1, e[:, 5 * P:6 * P], slice(P, 2 * P)))
                    for i, (kb, rhs, sl) in enumerate(plan):
                        first, last = i == 0, i == len(plan) - 1
                        nc.tensor.matmul(psc[:, sl], v_sb[:, kb, :], rhs,
                                         start=first, stop=last)
                        nc.tensor.matmul(psd[:, sl], ones, rhs,
                                         start=first, stop=last)

                    if pi % 2 == 0:
                        nc.scalar.copy(o_hs[:, q0:q0 + 2 * P], psc)
                    else:
                        nc.vector.tensor_copy(o_hs[:, q0:q0 + 2 * P], psc)
                    if pi % 2 == 1:
                        nc.vector.tensor_copy(
                            den_sb[:, (pi - 1) * 2 * P:(pi + 1) * 2 * P], psd2)
                    if pi % 4 == 3:
                        c0 = (pi - 3) * 2 * P
                        nc.sync.dma_start(
                            out_t[g][:, c0:c0 + 8 * P], o_hs[:, c0:c0 + 8 * P])

                nc.sync.dma_start(den[g], den_sb)
    nc.compile()
    return nc


def _np_reference(q, k, v, layer_idx):
    """Slow fallback for an even layer_idx (pure causal) - not the graded
    configuration, kept for functional completeness."""
    scale = 1.0 / math.sqrt(q.shape[-1])
    s = np.einsum("bhqd,bhkd->bhqk", q, k) * scale
    i = np.arange(s.shape[-2])[:, None]
    j = np.arange(s.shape[-1])[None, :]
    mask = j <= i
    if layer_idx % 2 != 0:
        mask &= j > i - WINDOW
    s = np.where(mask[None, None], s, np.float32(-1e9))
    s -= s.max(-1, keepdims=True)
    w = np.exp(s)
    w /= w.sum(-1, keepdims=True)
    ctx = np.einsum("bhqk,bhkd->bhqd", w, v)
    b, h, sq, d = q.shape
    return ctx.transpose(0, 2, 1, 3).reshape(b, sq, h * d).astype(np.float32)


def make_in_maps(q, k, v):
    qf = q.reshape(B * H, S, D)
    kf = k.reshape(B * H, S, D)
    vf = v.reshape(B * H, S, D)
    qT = np.ascontiguousarray(qf.transpose(0, 2, 1)).astype(np.float16)
    kT = np.ascontiguousarray(kf.transpose(0, 2, 1)).astype(np.float16)
    # [BH, S, D] -> [BH, P, NT, D]: tile index inner so each head-slice's
    # V loads as one contiguous DMA into a [P, NT, D] SBUF tile
    vt = np.ascontiguousarray(
        vf.reshape(B * H, NT, P, D).transpose(0, 2, 1, 3)).astype(np.float16)

    one = np.ones((P, P), np.float16)
    mhi = np.triu(one)        # valid kk <= q
    mlo = np.tril(one, -1)    # valid kk > q
    strip = np.concatenate([mlo, one, mlo, mhi, one, mhi],
                           axis=1).astype(np.float16)

    in_maps = []
    for c in range(NCORES):
        sl = slice(c * G, (c + 1) * G)
        in_maps.append({
            "qT": np.ascontiguousarray(qT[sl]),
            "kT": np.ascontiguousarray(kT[sl]),
            "v": np.ascontiguousarray(vt[sl]),
            "masks": strip,
        })
    return in_maps


def kernel(q, k, v, layer_idx, training):
    q = np.asarray(q, dtype=np.float32)
    k = np.asarray(k, dtype=np.float32)
    v = np.asarray(v, dtype=np.float32)
    li = int(layer_idx)
    if li % 2 == 0:
        return _np_reference(q, k, v, li)

    in_maps = make_in_maps(q, k, v)

    if "nc" not in _RUNNER_CACHE:
        _RUNNER_CACHE["nc"] = build_nc()
    nc = _RUNNER_CACHE["nc"]
    res = run_bass_kernel_spmd(nc, in_maps, core_ids=list(range(NCORES)))

    ctx_t = np.concatenate(
        [r["out_t"] for r in res.results], axis=0).astype(np.float32)
    den = np.concatenate([r["den"] for r in res.results], axis=0)
    ctx_t = ctx_t / den                       # [32, D, S] / [32, 1, S]
    out = ctx_t.reshape(B, H, D, S).transpose(0, 3, 1, 2).reshape(B, S, H * D)
    return np.ascontiguousarray(out.astype(np.float32))


# revision 44
# speedup vs baseline: 1.1277x; 1.0089x over previous
"""Banded causal attention (local window 256) for trn2, 8-core SPMD.

Problem: B=2, H=16, S=2048, D=128, layer_idx=1 (odd) -> mask = causal AND
(j > i - 256). Each query attends to at most 256 keys, so scores are only
computed on the key-blocks (of 128) that intersect each query tile's
window.

Sharding: B*H = 32 head-slices, 4 per core.  Each core computes its heads'
full banded attention independently; the host merges heads afterwards.

Per-core kernel, per query-tile pair (256 queries, key blocks r0..r3):
  - fp16 operands everywhere (11-bit mantissa = the TF32 rounding the PE
    applies to fp32 anyway; halves DMA and SBUF; full matmul rate at any
    free dim)
  - scores S_T[kk, q]: r1/r2 at N=256, boundary r0/r3 only their valid
    128-query half; exp on ACT with scale=1/sqrt(D) folded in, written
    into one [128, 768] P tile; one 768-wide 0/1 mask multiply split
    between DVE and GpSimd
  - ctx^T[d, q] and softmax denominator accumulate in PSUM via matmuls
    (lhsT = V tile / ones column)
  - unnormalized fp16 ctx^T and fp32 denom DMA'd out; host divides and
    merges heads
"""

import math
import os
import sys

import numpy as np

for _p in ("/root/.axon_site/_ro/trn_rl_repo", "/opt/trn_rl_repo"):
    if os.path.isdir(_p) and _p not in sys.path:
        sys.path.append(_p)

import concourse.bacc as bacc
import concourse.mybir as mybir
import concourse.tile as tile
from concourse.bass_utils import run_bass_kernel_spmd

F32 = mybir.dt.float32
F16 = mybir.dt.float16

B, H, S, D = 2, 16, 2048, 128
P = 128
NT = S // P           # 16 query/key tiles per head-slice
NCORES = 8
G = (B * H) // NCORES  # 4 head-slices per core
WINDOW = 256
SCALE = 1.0 / math.sqrt(D)

_RUNNER_CACHE = {}


def build_nc():
    nc = bacc.Bacc("TRN2", target_bir_lowering=False, debug=False)
    qT = nc.declare_dram_parameter("qT", [G, P, S], F16, isOutput=False)
    kT = nc.declare_dram_parameter("kT", [G, P, S], F16, isOutput=False)
    # v is host-pre-tiled to [G, P, NT, D] (partition dim first) so the whole
    # head-slice loads as one fully-contiguous DMA
    v = nc.declare_dram_parameter("v", [G, P, NT, D], F16, isOutput=False)
    masks = nc.declare_dram_parameter("masks", [P, 6 * P], F16, isOutput=False)
    out_t = nc.declare_dram_parameter("out_t", [G, P, S], F16, isOutput=True)
    den = nc.declare_dram_parameter("den", [G, 1, S], F32, isOutput=True)

    EXP = mybir.ActivationFunctionType.Exp
    MUL = mybir.AluOpType.mult

    with tile.TileContext(nc) as tc:
        with (
            tc.tile_pool(name="const", bufs=1) as constp,
            tc.tile_pool(name="kv", bufs=3) as kvp,
            tc.tile_pool(name="pt", bufs=4) as ptp,
            tc.tile_pool(name="ps", bufs=2, space="PSUM") as psp,
            tc.tile_pool(name="ps1", bufs=1, space="PSUM") as psp1,
        ):
            # mask strip, columns [Mlo | 1 | Mlo | Mhi | 1 | Mhi] matching the
            # P-tile layout [r0(128) | r1(256) | r2(256) | r3(128)]
            strip = constp.tile([P, 6 * P], F16, tag="strip")
            nc.sync.dma_start(strip, masks.ap())
            ones = constp.tile([P, 1], F16, tag="ones")
            nc.vector.memset(ones, 1.0)

            for g in range(G):
                kt_sb = kvp.tile([P, NT, P], F16, tag="kt")
                qt_sb = kvp.tile([P, NT, P], F16, tag="qt")
                v_sb = kvp.tile([P, NT, D], F16, tag="v")
                # input loads: HWDGE(SP) for K/Q, SWDGE for V; halves so
                # pair-0 compute starts before the whole head-slice lands
                kt_d = kT[g].rearrange("d (n p) -> d n p", p=P)
                qt_d = qT[g].rearrange("d (n p) -> d n p", p=P)
                # tiny head chunk (2 tiles) so pair-0's matmuls start as soon
                # as ~128 KB lands, then the rest in two waves
                hn = NT // 2
                nc.sync.dma_start(kt_sb[:, 0:2, :], kt_d[:, 0:2, :])
                nc.sync.dma_start(qt_sb[:, 0:2, :], qt_d[:, 0:2, :])
                nc.gpsimd.dma_start(v_sb[:, 0:2, :], v[g][:, 0:2, :])
                nc.sync.dma_start(kt_sb[:, 2:hn, :], kt_d[:, 2:hn, :])
                nc.sync.dma_start(qt_sb[:, 2:hn, :], qt_d[:, 2:hn, :])
                nc.gpsimd.dma_start(v_sb[:, 2:hn, :], v[g][:, 2:hn, :])
                nc.sync.dma_start(kt_sb[:, hn:NT, :], kt_d[:, hn:NT, :])
                nc.sync.dma_start(qt_sb[:, hn:NT, :], qt_d[:, hn:NT, :])
                nc.gpsimd.dma_start(v_sb[:, hn:NT, :], v[g][:, hn:NT, :])
                den_sb = kvp.tile([1, S], F32, tag="den")
                o_hs = kvp.tile([P, S], F16, tag="ohs")

                for pi in range(NT // 2):
                    t = 2 * pi            # first q-tile of the pair
                    q0 = t * P            # absolute first query column
                    roles = [r for r in range(4) if t - 2 + r >= 0]
                    qs = qt_sb[:, t:t + 2, :].rearrange("d a b -> d (a b)")

                    ps12 = psp.tile([P, 4 * P], F32, tag="ps12")
                    psc = psp.tile([P, 2 * P], F32, tag="psc")
                    if pi % 2 == 0:
                        psd2 = psp.tile([1, 4 * P], F32, tag="psd", name="psd2")
                    psd = psd2[:, (pi % 2) * 2 * P:(pi % 2 + 1) * 2 * P]
                    ps0 = (psp1.tile([P, P], F32, tag="ps0", name="ps0")
                           if 0 in roles else None)
                    ps3 = psp1.tile([P, P], F32, tag="ps3", name="ps3")

                    # score matmuls; boundary blocks only their valid q-half
                    if 0 in roles:
                        nc.tensor.matmul(ps0, kt_sb[:, t - 2, :], qs[:, 0:P],
                                         start=True, stop=True)
                    if 1 in roles:
                        nc.tensor.matmul(ps12[:, 0:2 * P], kt_sb[:, t - 1, :],
                                         qs, start=True, stop=True)
                    nc.tensor.matmul(ps12[:, 2 * P:4 * P], kt_sb[:, t, :],
                                     qs, start=True, stop=True)
                    nc.tensor.matmul(ps3, kt_sb[:, t + 1, :], qs[:, P:2 * P],
                                     start=True, stop=True)

                    # P tile [r0 | r1 | r2 | r3] = [128 | 256 | 256 | 128]
                    e = ptp.tile([P, 6 * P], F16, tag="e")
                    if 0 in roles:
                        nc.scalar.activation(e[:, 0:P], ps0, EXP, scale=SCALE)
                    if 1 in roles:
                        nc.scalar.activation(e[:, P:5 * P], ps12, EXP,
                                             scale=SCALE)
                    else:
                        nc.scalar.activation(e[:, 3 * P:5 * P],
                                             ps12[:, 2 * P:4 * P], EXP,
                                             scale=SCALE)
                    nc.scalar.activation(e[:, 5 * P:6 * P], ps3, EXP,
                                         scale=SCALE)

                    # 0/1 mask multiplies on DVE (two halves for finer deps)
                    if 0 in roles:
                        nc.vector.tensor_tensor(
                            e[:, 0:3 * P], e[:, 0:3 * P], strip[:, 0:3 * P],
                            MUL)
                    nc.vector.tensor_tensor(
                        e[:, 3 * P:6 * P], e[:, 3 * P:6 * P],
                        strip[:, 3 * P:6 * P], MUL)

                    # ctx^T + denominator accumulation (full-width roles
                    # first so PSUM pending-zero state stays uniform)
                    plan = []
                    if 1 in roles:
                        plan.append((t - 1, e[:, P:3 * P], slice(0, 2 * P)))
                    plan.append((t, e[:, 3 * P:5 * P], slice(0, 2 * P)))
                    if 0 in roles:
                        plan.append((t - 2, e[:, 0:P], slice(0, P)))
                    plan.append((t + 1, e[:, 5 * P:6 * P], slice(P, 2 * P)))
                    for i, (kb, rhs, sl) in enumerate(plan):
                        first, last = i == 0, i == len(plan) - 1
                        nc.tensor.matmul(psc[:, sl], v_sb[:, kb, :], rhs,
                                         start=first, stop=last)
                        nc.tensor.matmul(psd[:, sl], ones, rhs,
                                         start=first, stop=last)

                    if pi % 2 == 0:
                        nc.scalar.copy(o_hs[:, q0:q0 + 2 * P], psc)
                    else:
                        nc.vector.tensor_copy(o_hs[:, q0:q0 + 2 * P], psc)
                    if pi % 2 == 1:
                        nc.vector.tensor_copy(
                            den_sb[:, (pi - 1) * 2 * P:(pi + 1) * 2 * P], psd2)
                    if pi % 4 == 3:
                        c0 = (pi - 3) * 2 * P
                        nc.scalar.dma_start(
                            out_t[g][:, c0:c0 + 8 * P], o_hs[:, c0:c0 + 8 * P])

                nc.scalar.dma_start(den[g], den_sb)
    nc.compile()
    return nc


def _np_reference(q, k, v, layer_idx):
    """Slow fallback for an even layer_idx (pure causal) - not the graded
    configuration, kept for functional completeness."""
    scale = 1.0 / math.sqrt(q.shape[-1])
    s = np.einsum("bhqd,bhkd->bhqk", q, k) * scale
    i = np.arange(s.shape[-2])[:, None]
    j = np.arange(s.shape[-1])[None, :]
    mask = j <= i
    if layer_idx % 2 != 0:
        mask &= j > i - WINDOW
    s = np.where(mask[None, None], s, np.float32(-1e9))
    s -= s.max(-1, keepdims=True)
    w = np.exp(s)
    w /= w.sum(-1, keepdims=True)
    ctx = np.einsum("bhqk,bhkd->bhqd", w, v)
    b, h, sq, d = q.shape
    return ctx.transpose(0, 2, 1, 3).reshape(b, sq, h * d).astype(np.float32)


def make_in_maps(q, k, v):
    qf = q.reshape(B * H, S, D)
    kf = k.reshape(B * H, S, D)
    vf = v.reshape(B * H, S, D)
    qT = np.ascontiguousarray(qf.transpose(0, 2, 1)).astype(np.float16)
    kT = np.ascontiguousarray(kf.transpose(0, 2, 1)).astype(np.float16)
    # [BH, S, D] -> [BH, P, NT, D]: tile index inner so each head-slice's
    # V loads as one contiguous DMA into a [P, NT, D] SBUF tile
    vt = np.ascontiguousarray(
        vf.reshape(B * H, NT, P, D).transpose(0, 2, 1, 3)).astype(np.float16)

    one = np.ones((P, P), np.float16)
    mhi = np.triu(one)        # valid kk <= q
    mlo = np.tril(one, -1)    # valid kk > q
    strip = np.concatenate([mlo, one, mlo, mhi, one, mhi],
                           axis=1).astype(np.float16)

    in_maps = []
    for c in range(NCORES):
        sl = slice(c * G, (c + 1) * G)
        in_maps.append({
            "qT": np.ascontiguousarray(qT[sl]),
            "kT": np.ascontiguousarray(kT[sl]),
            "v": np.ascontiguousarray(vt[sl]),
            "masks": strip,
        })
    return in_maps


def kernel(q, k, v, layer_idx, training):
    q = np.asarray(q, dtype=np.float32)
    k = np.asarray(k, dtype=np.float32)
    v = np.asarray(v, dtype=np.float32)
    li = int(layer_idx)
    if li % 2 == 0:
        return _np_reference(q, k, v, li)

    in_maps = make_in_maps(q, k, v)

    if "nc" not in _RUNNER_CACHE:
        _RUNNER_CACHE["nc"] = build_nc()
    nc = _RUNNER_CACHE["nc"]
    res = run_bass_kernel_spmd(nc, in_maps, core_ids=list(range(NCORES)))

    ctx_t = np.concatenate(
        [r["out_t"] for r in res.results], axis=0).astype(np.float32)
    den = np.concatenate([r["den"] for r in res.results], axis=0)
    ctx_t = ctx_t / den                       # [32, D, S] / [32, 1, S]
    out = ctx_t.reshape(B, H, D, S).transpose(0, 3, 1, 2).reshape(B, S, H * D)
    return np.ascontiguousarray(out.astype(np.float32))


# revision 46
# speedup vs baseline: 1.3304x; 1.1797x over previous
"""Banded causal attention (local window 256) for trn2, 8-core SPMD.

Problem: B=2, H=16, S=2048, D=128, layer_idx=1 (odd) -> mask = causal AND
(j > i - 256). Each query attends to at most 256 keys, so scores are only
computed on the key-blocks (of 128) that intersect each query tile's
window.

Sharding: B*H = 32 head-slices, 4 per core.  Each core computes its heads'
full banded attention independently; the host merges heads afterwards.

Per-core kernel, per query-tile pair (256 queries, key blocks r0..r3):
  - fp16 operands everywhere (11-bit mantissa = the TF32 rounding the PE
    applies to fp32 anyway; halves DMA and SBUF; full matmul rate at any
    free dim)
  - scores S_T[kk, q]: r1/r2 at N=256, boundary r0/r3 only their valid
    128-query half; exp on ACT with scale=1/sqrt(D) folded in, written
    into one [128, 768] P tile; one 768-wide 0/1 mask multiply split
    between DVE and GpSimd
  - ctx^T[d, q] and softmax denominator accumulate in PSUM via matmuls
    (lhsT = V tile / ones column)
  - unnormalized fp16 ctx^T and fp32 denom DMA'd out; host divides and
    merges heads
"""

import math
import os
import sys

import numpy as np

for _p in ("/root/.axon_site/_ro/trn_rl_repo", "/opt/trn_rl_repo"):
    if os.path.isdir(_p) and _p not in sys.path:
        sys.path.append(_p)

import concourse.bacc as bacc
import concourse.mybir as mybir
import concourse.tile as tile
from concourse.bass_utils import run_bass_kernel_spmd

F32 = mybir.dt.float32
F16 = mybir.dt.float16

B, H, S, D = 2, 16, 2048, 128
P = 128
NT = S // P           # 16 query/key tiles per head-slice
NCORES = 8
G = (B * H) // NCORES  # 4 head-slices per core
WINDOW = 256
SCALE = 1.0 / math.sqrt(D)

_RUNNER_CACHE = {}


def build_nc():
    nc = bacc.Bacc("TRN2", target_bir_lowering=False, debug=False)
    qT = nc.declare_dram_parameter("qT", [G, P, S], F16, isOutput=False)
    kT = nc.declare_dram_parameter("kT", [G, P, S], F16, isOutput=False)
    # v is host-pre-tiled to [G, P, NT, D] (partition dim first) so the whole
    # head-slice loads as one fully-contiguous DMA
    v = nc.declare_dram_parameter("v", [G, P, NT, D], F16, isOutput=False)
    masks = nc.declare_dram_parameter("masks", [P, 6 * P], F16, isOutput=False)
    out_t = nc.declare_dram_parameter("out_t", [G, P, S], F16, isOutput=True)
    den = nc.declare_dram_parameter("den", [G, 1, S], F32, isOutput=True)

    EXP = mybir.ActivationFunctionType.Exp
    MUL = mybir.AluOpType.mult

    with tile.TileContext(nc) as tc:
        with (
            tc.tile_pool(name="const", bufs=1) as constp,
            tc.tile_pool(name="kv", bufs=3) as kvp,
            tc.tile_pool(name="pt", bufs=6) as ptp,
            tc.tile_pool(name="ps", bufs=2, space="PSUM") as psp,
            tc.tile_pool(name="ps1", bufs=1, space="PSUM") as psp1,
        ):
            # mask strip, columns [Mlo | 1 | Mlo | Mhi | 1 | Mhi] matching the
            # P-tile layout [r0(128) | r1(256) | r2(256) | r3(128)]
            strip = constp.tile([P, 6 * P], F16, tag="strip")
            nc.sync.dma_start(strip, masks.ap())
            ones = constp.tile([P, 1], F16, tag="ones")
            nc.vector.memset(ones, 1.0)

            for g in range(G):
                kt_sb = kvp.tile([P, NT, P], F16, tag="kt")
                qt_sb = kvp.tile([P, NT, P], F16, tag="qt")
                v_sb = kvp.tile([P, NT, D], F16, tag="v")
                # input loads: HWDGE(SP) for K/Q, SWDGE for V; halves so
                # pair-0 compute starts before the whole head-slice lands
                kt_d = kT[g].rearrange("d (n p) -> d n p", p=P)
                qt_d = qT[g].rearrange("d (n p) -> d n p", p=P)
                # tiny head chunk (2 tiles) so pair-0's matmuls start as soon
                # as ~128 KB lands, then the rest in two waves
                hn = NT // 2
                nc.sync.dma_start(kt_sb[:, 0:2, :], kt_d[:, 0:2, :])
                nc.sync.dma_start(qt_sb[:, 0:2, :], qt_d[:, 0:2, :])
                nc.gpsimd.dma_start(v_sb[:, 0:2, :], v[g][:, 0:2, :])
                nc.sync.dma_start(kt_sb[:, 2:hn, :], kt_d[:, 2:hn, :])
                nc.sync.dma_start(qt_sb[:, 2:hn, :], qt_d[:, 2:hn, :])
                nc.gpsimd.dma_start(v_sb[:, 2:hn, :], v[g][:, 2:hn, :])
                nc.sync.dma_start(kt_sb[:, hn:NT, :], kt_d[:, hn:NT, :])
                nc.sync.dma_start(qt_sb[:, hn:NT, :], qt_d[:, hn:NT, :])
                nc.gpsimd.dma_start(v_sb[:, hn:NT, :], v[g][:, hn:NT, :])
                den_sb = kvp.tile([1, S], F32, tag="den")
                o_hs = kvp.tile([P, S], F16, tag="ohs")

                for pi in range(NT // 2):
                    t = 2 * pi            # first q-tile of the pair
                    q0 = t * P            # absolute first query column
                    roles = [r for r in range(4) if t - 2 + r >= 0]
                    qs = qt_sb[:, t:t + 2, :].rearrange("d a b -> d (a b)")

                    ps12 = psp.tile([P, 4 * P], F32, tag="ps12")
                    psc = psp.tile([P, 2 * P], F32, tag="psc")
                    if pi % 2 == 0:
                        psd2 = psp.tile([1, 4 * P], F32, tag="psd", name="psd2")
                    psd = psd2[:, (pi % 2) * 2 * P:(pi % 2 + 1) * 2 * P]
                    ps0 = (psp1.tile([P, P], F32, tag="ps0", name="ps0")
                           if 0 in roles else None)
                    ps3 = psp1.tile([P, P], F32, tag="ps3", name="ps3")

                    # score matmuls; boundary blocks only their valid q-half
                    if 0 in roles:
                        nc.tensor.matmul(ps0, kt_sb[:, t - 2, :], qs[:, 0:P],
                                         start=True, stop=True)
                    if 1 in roles:
                        nc.tensor.matmul(ps12[:, 0:2 * P], kt_sb[:, t - 1, :],
                                         qs, start=True, stop=True)
                    nc.tensor.matmul(ps12[:, 2 * P:4 * P], kt_sb[:, t, :],
                                     qs, start=True, stop=True)
                    nc.tensor.matmul(ps3, kt_sb[:, t + 1, :], qs[:, P:2 * P],
                                     start=True, stop=True)

                    # P tile [r0 | r1 | r2 | r3] = [128 | 256 | 256 | 128]
                    e = ptp.tile([P, 6 * P], F16, tag="e")
                    if 0 in roles:
                        nc.scalar.activation(e[:, 0:P], ps0, EXP, scale=SCALE)
                    if 1 in roles:
                        nc.scalar.activation(e[:, P:5 * P], ps12, EXP,
                                             scale=SCALE)
                    else:
                        nc.scalar.activation(e[:, 3 * P:5 * P],
                                             ps12[:, 2 * P:4 * P], EXP,
                                             scale=SCALE)
                    nc.scalar.activation(e[:, 5 * P:6 * P], ps3, EXP,
                                         scale=SCALE)

                    # 0/1 mask multiplies on DVE (two halves for finer deps)
                    if 0 in roles:
                        nc.vector.tensor_tensor(
                            e[:, 0:3 * P], e[:, 0:3 * P], strip[:, 0:3 * P],
                            MUL)
                    nc.vector.tensor_tensor(
                        e[:, 3 * P:6 * P], e[:, 3 * P:6 * P],
                        strip[:, 3 * P:6 * P], MUL)

                    # ctx^T + denominator accumulation (full-width roles
                    # first so PSUM pending-zero state stays uniform)
                    plan = []
                    if 1 in roles:
                        plan.append((t - 1, e[:, P:3 * P], slice(0, 2 * P)))
                    plan.append((t, e[:, 3 * P:5 * P], slice(0, 2 * P)))
                    if 0 in roles:
                        plan.append((t - 2, e[:, 0:P], slice(0, P)))
                    plan.append((t + 1, e[:, 5 * P:6 * P], slice(P, 2 * P)))
                    # ctx matmuls first, then all denominator matmuls: the
                    # den group shares one stationary `ones` operand, so
                    # grouping avoids alternating weight reloads every matmul
                    for i, (kb, rhs, sl) in enumerate(plan):
                        first, last = i == 0, i == len(plan) - 1
                        nc.tensor.matmul(psc[:, sl], v_sb[:, kb, :], rhs,
                                         start=first, stop=last)
                    for i, (kb, rhs, sl) in enumerate(plan):
                        first, last = i == 0, i == len(plan) - 1
                        nc.tensor.matmul(psd[:, sl], ones, rhs,
                                         start=first, stop=last)

                    if pi % 2 == 0:
                        nc.scalar.copy(o_hs[:, q0:q0 + 2 * P], psc)
                    else:
                        nc.vector.tensor_copy(o_hs[:, q0:q0 + 2 * P], psc)
                    if pi % 2 == 1:
                        nc.vector.tensor_copy(
                            den_sb[:, (pi - 1) * 2 * P:(pi + 1) * 2 * P], psd2)
                    if pi % 4 == 3:
                        c0 = (pi - 3) * 2 * P
                        nc.scalar.dma_start(
                            out_t[g][:, c0:c0 + 8 * P], o_hs[:, c0:c0 + 8 * P])

                nc.scalar.dma_start(den[g], den_sb)
    nc.compile()
    return nc


def _np_reference(q, k, v, layer_idx):
    """Slow fallback for an even layer_idx (pure causal) - not the graded
    configuration, kept for functional completeness."""
    scale = 1.0 / math.sqrt(q.shape[-1])
    s = np.einsum("bhqd,bhkd->bhqk", q, k) * scale
    i = np.arange(s.shape[-2])[:, None]
    j = np.arange(s.shape[-1])[None, :]
    mask = j <= i
    if layer_idx % 2 != 0:
        mask &= j > i - WINDOW
    s = np.where(mask[None, None], s, np.float32(-1e9))
    s -= s.max(-1, keepdims=True)
    w = np.exp(s)
    w /= w.sum(-1, keepdims=True)
    ctx = np.einsum("bhqk,bhkd->bhqd", w, v)
    b, h, sq, d = q.shape
    return ctx.transpose(0, 2, 1, 3).reshape(b, sq, h * d).astype(np.float32)


def make_in_maps(q, k, v):
    qf = q.reshape(B * H, S, D)
    kf = k.reshape(B * H, S, D)
    vf = v.reshape(B * H, S, D)
    qT = np.ascontiguousarray(qf.transpose(0, 2, 1)).astype(np.float16)
    kT = np.ascontiguousarray(kf.transpose(0, 2, 1)).astype(np.float16)
    # [BH, S, D] -> [BH, P, NT, D]: tile index inner so each head-slice's
    # V loads as one contiguous DMA into a [P, NT, D] SBUF tile
    vt = np.ascontiguousarray(
        vf.reshape(B * H, NT, P, D).transpose(0, 2, 1, 3)).astype(np.float16)

    one = np.ones((P, P), np.float16)
    mhi = np.triu(one)        # valid kk <= q
    mlo = np.tril(one, -1)    # valid kk > q
    strip = np.concatenate([mlo, one, mlo, mhi, one, mhi],
                           axis=1).astype(np.float16)

    in_maps = []
    for c in range(NCORES):
        sl = slice(c * G, (c + 1) * G)
        in_maps.append({
            "qT": np.ascontiguousarray(qT[sl]),
            "kT": np.ascontiguousarray(kT[sl]),
            "v": np.ascontiguousarray(vt[sl]),
            "masks": strip,
        })
    return in_maps


def kernel(q, k, v, layer_idx, training):
    q = np.asarray(q, dtype=np.float32)
    k = np.asarray(k, dtype=np.float32)
    v = np.asarray(v, dtype=np.float32)
    li = int(layer_idx)
    if li % 2 == 0:
        return _np_reference(q, k, v, li)

    in_maps = make_in_maps(q, k, v)

    if "nc" not in _RUNNER_CACHE:
        _RUNNER_CACHE["nc"] = build_nc()
    nc = _RUNNER_CACHE["nc"]
    res = run_bass_kernel_spmd(nc, in_maps, core_ids=list(range(NCORES)))

    ctx_t = np.concatenate(
        [r["out_t"] for r in res.results], axis=0).astype(np.float32)
    den = np.concatenate([r["den"] for r in res.results], axis=0)
    ctx_t = ctx_t / den                       # [32, D, S] / [32, 1, S]
    out = ctx_t.reshape(B, H, D, S).transpose(0, 3, 1, 2).reshape(B, S, H * D)
    return np.ascontiguousarray(out.astype(np.float32))


# revision 48
# speedup vs baseline: 1.3959x; 1.0493x over previous
"""Banded causal attention (local window 256) for trn2, 8-core SPMD.

Problem: B=2, H=16, S=2048, D=128, layer_idx=1 (odd) -> mask = causal AND
(j > i - 256). Each query attends to at most 256 keys, so scores are only
computed on the key-blocks (of 128) that intersect each query tile's
window.

Sharding: B*H = 32 head-slices, 4 per core.  Each core computes its heads'
full banded attention independently; the host merges heads afterwards.

Per-core kernel, per query-tile pair (256 queries, key blocks r0..r3):
  - fp16 operands everywhere (11-bit mantissa = the TF32 rounding the PE
    applies to fp32 anyway; halves DMA and SBUF; full matmul rate at any
    free dim)
  - scores S_T[kk, q]: r1/r2 at N=256, boundary r0/r3 only their valid
    128-query half; exp on ACT with scale=1/sqrt(D) folded in, written
    into one [128, 768] P tile; one 768-wide 0/1 mask multiply split
    between DVE and GpSimd
  - ctx^T[d, q] and softmax denominator accumulate in PSUM via matmuls
    (lhsT = V tile / ones column)
  - unnormalized fp16 ctx^T and fp32 denom DMA'd out; host divides and
    merges heads
"""

import math
import os
import sys

import numpy as np

for _p in ("/root/.axon_site/_ro/trn_rl_repo", "/opt/trn_rl_repo"):
    if os.path.isdir(_p) and _p not in sys.path:
        sys.path.append(_p)

import concourse.bacc as bacc
import concourse.mybir as mybir
import concourse.tile as tile
from concourse.bass_utils import run_bass_kernel_spmd

F32 = mybir.dt.float32
F16 = mybir.dt.float16

B, H, S, D = 2, 16, 2048, 128
P = 128
NT = S // P           # 16 query/key tiles per head-slice
NCORES = 8
G = (B * H) // NCORES  # 4 head-slices per core
WINDOW = 256
SCALE = 1.0 / math.sqrt(D)

_RUNNER_CACHE = {}


def build_nc():
    nc = bacc.Bacc("TRN2", target_bir_lowering=False, debug=False)
    qT = nc.declare_dram_parameter("qT", [G, P, S], F16, isOutput=False)
    kT = nc.declare_dram_parameter("kT", [G, P, S], F16, isOutput=False)
    # v is host-pre-tiled to [G, P, NT, D] (partition dim first) so the whole
    # head-slice loads as one fully-contiguous DMA
    v = nc.declare_dram_parameter("v", [G, P, NT, D], F16, isOutput=False)
    masks = nc.declare_dram_parameter("masks", [P, 6 * P], F16, isOutput=False)
    out_t = nc.declare_dram_parameter("out_t", [G, P, S], F16, isOutput=True)
    den = nc.declare_dram_parameter("den", [G, 1, S], F32, isOutput=True)

    EXP = mybir.ActivationFunctionType.Exp
    MUL = mybir.AluOpType.mult

    with tile.TileContext(nc) as tc:
        with (
            tc.tile_pool(name="const", bufs=1) as constp,
            tc.tile_pool(name="kv", bufs=3) as kvp,
            tc.tile_pool(name="pt", bufs=6) as ptp,
            tc.tile_pool(name="ps", bufs=2, space="PSUM") as psp,
            tc.tile_pool(name="ps1", bufs=1, space="PSUM") as psp1,
        ):
            # mask strip, columns [Mlo | 1 | Mlo | Mhi | 1 | Mhi] matching the
            # P-tile layout [r0(128) | r1(256) | r2(256) | r3(128)]
            strip = constp.tile([P, 6 * P], F16, tag="strip")
            nc.sync.dma_start(strip, masks.ap())
            ones = constp.tile([P, 1], F16, tag="ones")
            nc.vector.memset(ones, 1.0)

            for g in range(G):
                kt_sb = kvp.tile([P, NT, P], F16, tag="kt")
                qt_sb = kvp.tile([P, NT, P], F16, tag="qt")
                v_sb = kvp.tile([P, NT, D], F16, tag="v")
                # input loads: HWDGE(SP) for K/Q, SWDGE for V; halves so
                # pair-0 compute starts before the whole head-slice lands
                kt_d = kT[g].rearrange("d (n p) -> d n p", p=P)
                qt_d = qT[g].rearrange("d (n p) -> d n p", p=P)
                # tiny head chunk (2 tiles) so pair-0's matmuls start as soon
                # as ~128 KB lands, then the rest in two waves
                hn = NT // 2
                nc.sync.dma_start(kt_sb[:, 0:2, :], kt_d[:, 0:2, :])
                nc.sync.dma_start(qt_sb[:, 0:2, :], qt_d[:, 0:2, :])
                nc.gpsimd.dma_start(v_sb[:, 0:2, :], v[g][:, 0:2, :])
                nc.sync.dma_start(kt_sb[:, 2:6, :], kt_d[:, 2:6, :])
                nc.sync.dma_start(qt_sb[:, 2:6, :], qt_d[:, 2:6, :])
                nc.gpsimd.dma_start(v_sb[:, 2:6, :], v[g][:, 2:6, :])
                nc.sync.dma_start(kt_sb[:, 6:hn, :], kt_d[:, 6:hn, :])
                nc.sync.dma_start(qt_sb[:, 6:hn, :], qt_d[:, 6:hn, :])
                nc.gpsimd.dma_start(v_sb[:, 6:hn, :], v[g][:, 6:hn, :])
                nc.sync.dma_start(kt_sb[:, hn:NT, :], kt_d[:, hn:NT, :])
                nc.sync.dma_start(qt_sb[:, hn:NT, :], qt_d[:, hn:NT, :])
                nc.gpsimd.dma_start(v_sb[:, hn:NT, :], v[g][:, hn:NT, :])
                den_sb = kvp.tile([1, S], F32, tag="den")
                o_hs = kvp.tile([P, S], F16, tag="ohs")

                for pi in range(NT // 2):
                    t = 2 * pi            # first q-tile of the pair
                    q0 = t * P            # absolute first query column
                    roles = [r for r in range(4) if t - 2 + r >= 0]
                    qs = qt_sb[:, t:t + 2, :].rearrange("d a b -> d (a b)")

                    ps12 = psp.tile([P, 4 * P], F32, tag="ps12")
                    psc = psp.tile([P, 2 * P], F32, tag="psc")
                    if pi % 2 == 0:
                        psd2 = psp.tile([1, 4 * P], F32, tag="psd", name="psd2")
                    psd = psd2[:, (pi % 2) * 2 * P:(pi % 2 + 1) * 2 * P]
                    ps0 = (psp1.tile([P, P], F32, tag="ps0", name="ps0")
                           if 0 in roles else None)
                    ps3 = psp1.tile([P, P], F32, tag="ps3", name="ps3")

                    # score matmuls; boundary blocks only their valid q-half
                    if 0 in roles:
                        nc.tensor.matmul(ps0, kt_sb[:, t - 2, :], qs[:, 0:P],
                                         start=True, stop=True)
                    if 1 in roles:
                        nc.tensor.matmul(ps12[:, 0:2 * P], kt_sb[:, t - 1, :],
                                         qs, start=True, stop=True)
                    nc.tensor.matmul(ps12[:, 2 * P:4 * P], kt_sb[:, t, :],
                                     qs, start=True, stop=True)
                    nc.tensor.matmul(ps3, kt_sb[:, t + 1, :], qs[:, P:2 * P],
                                     start=True, stop=True)

                    # P tile [r0 | r1 | r2 | r3] = [128 | 256 | 256 | 128]
                    e = ptp.tile([P, 6 * P], F16, tag="e")
                    if 0 in roles:
                        nc.scalar.activation(e[:, 0:P], ps0, EXP, scale=SCALE)
                    if 1 in roles:
                        nc.scalar.activation(e[:, P:5 * P], ps12, EXP,
                                             scale=SCALE)
                    else:
                        nc.scalar.activation(e[:, 3 * P:5 * P],
                                             ps12[:, 2 * P:4 * P], EXP,
                                             scale=SCALE)
                    nc.scalar.activation(e[:, 5 * P:6 * P], ps3, EXP,
                                         scale=SCALE)

                    # 0/1 mask multiplies on DVE (two halves for finer deps)
                    if 0 in roles:
                        nc.vector.tensor_tensor(
                            e[:, 0:3 * P], e[:, 0:3 * P], strip[:, 0:3 * P],
                            MUL)
                    nc.vector.tensor_tensor(
                        e[:, 3 * P:6 * P], e[:, 3 * P:6 * P],
                        strip[:, 3 * P:6 * P], MUL)

                    # ctx^T + denominator accumulation (full-width roles
                    # first so PSUM pending-zero state stays uniform)
                    plan = []
                    if 1 in roles:
                        plan.append((t - 1, e[:, P:3 * P], slice(0, 2 * P)))
                    plan.append((t, e[:, 3 * P:5 * P], slice(0, 2 * P)))
                    if 0 in roles:
                        plan.append((t - 2, e[:, 0:P], slice(0, P)))
                    plan.append((t + 1, e[:, 5 * P:6 * P], slice(P, 2 * P)))
                    # ctx matmuls first, then all denominator matmuls: the
                    # den group shares one stationary `ones` operand, so
                    # grouping avoids alternating weight reloads every matmul
                    for i, (kb, rhs, sl) in enumerate(plan):
                        first, last = i == 0, i == len(plan) - 1
                        nc.tensor.matmul(psc[:, sl], v_sb[:, kb, :], rhs,
                                         start=first, stop=last)
                    for i, (kb, rhs, sl) in enumerate(plan):
                        first, last = i == 0, i == len(plan) - 1
                        nc.tensor.matmul(psd[:, sl], ones, rhs,
                                         start=first, stop=last)

                    nc.vector.tensor_copy(o_hs[:, q0:q0 + 2 * P], psc)
                    if pi % 2 == 1:
                        nc.vector.tensor_copy(
                            den_sb[:, (pi - 1) * 2 * P:(pi + 1) * 2 * P], psd2)
                        c0 = (pi - 1) * 2 * P
                        nc.scalar.dma_start(
                            out_t[g][:, c0:c0 + 4 * P], o_hs[:, c0:c0 + 4 * P])

                nc.scalar.dma_start(den[g], den_sb)
    nc.compile()
    return nc


def _np_reference(q, k, v, layer_idx):
    """Slow fallback for an even layer_idx (pure causal) - not the graded
    configuration, kept for functional completeness."""
    scale = 1.0 / math.sqrt(q.shape[-1])
    s = np.einsum("bhqd,bhkd->bhqk", q, k) * scale
    i = np.arange(s.shape[-2])[:, None]
    j = np.arange(s.shape[-1])[None, :]
    mask = j <= i
    if layer_idx % 2 != 0:
        mask &= j > i - WINDOW
    s = np.where(mask[None, None], s, np.float32(-1e9))
    s -= s.max(-1, keepdims=True)
    w = np.exp(s)
    w /= w.sum(-1, keepdims=True)
    ctx = np.einsum("bhqk,bhkd->bhqd", w, v)
    b, h, sq, d = q.shape
    return ctx.transpose(0, 2, 1, 3).reshape(b, sq, h * d).astype(np.float32)


def make_in_maps(q, k, v):
    qf = q.reshape(B * H, S, D)
    kf = k.reshape(B * H, S, D)
    vf = v.reshape(B * H, S, D)
    qT = np.ascontiguousarray(qf.transpose(0, 2, 1)).astype(np.float16)
    kT = np.ascontiguousarray(kf.transpose(0, 2, 1)).astype(np.float16)
    # [BH, S, D] -> [BH, P, NT, D]: tile index inner so each head-slice's
    # V loads as one contiguous DMA into a [P, NT, D] SBUF tile
    vt = np.ascontiguousarray(
        vf.reshape(B * H, NT, P, D).transpose(0, 2, 1, 3)).astype(np.float16)

    one = np.ones((P, P), np.float16)
    mhi = np.triu(one)        # valid kk <= q
    mlo = np.tril(one, -1)    # valid kk > q
    strip = np.concatenate([mlo, one, mlo, mhi, one, mhi],
                           axis=1).astype(np.float16)

    in_maps = []
    for c in range(NCORES):
        sl = slice(c * G, (c + 1) * G)
        in_maps.append({
            "qT": np.ascontiguousarray(qT[sl]),
            "kT": np.ascontiguousarray(kT[sl]),
            "v": np.ascontiguousarray(vt[sl]),
            "masks": strip,
        })
    return in_maps


def kernel(q, k, v, layer_idx, training):
    q = np.asarray(q, dtype=np.float32)
    k = np.asarray(k, dtype=np.float32)
    v = np.asarray(v, dtype=np.float32)
    li = int(layer_idx)
    if li % 2 == 0:
        return _np_reference(q, k, v, li)

    in_maps = make_in_maps(q, k, v)

    if "nc" not in _RUNNER_CACHE:
        _RUNNER_CACHE["nc"] = build_nc()
    nc = _RUNNER_CACHE["nc"]
    res = run_bass_kernel_spmd(nc, in_maps, core_ids=list(range(NCORES)))

    ctx_t = np.concatenate(
        [r["out_t"] for r in res.results], axis=0).astype(np.float32)
    den = np.concatenate([r["den"] for r in res.results], axis=0)
    ctx_t = ctx_t / den                       # [32, D, S] / [32, 1, S]
    out = ctx_t.reshape(B, H, D, S).transpose(0, 3, 1, 2).reshape(B, S, H * D)
    return np.ascontiguousarray(out.astype(np.float32))
